# revision 71
# baseline (speedup 1.0000x reference)
"""Multi-head attention (B=2, S=2048, D=1024, H=16) on 8 trn2 NeuronCores.

Sharding: core c handles batch b = c//4 and head-group g = c%4 (4 heads).
Megatron-style: Wq/Wk/Wv column-split, Wo row-split; host sums the 4 partial
outputs per batch and adds bo.

Per-core math (all matmuls bf16 in, fp32 accumulate):
  phase 1: qT/kT = (W^T x^T) in [256, S] layout, v = x W in [S, 260] layout
           (v augmented with a ones column per head -> softmax row sums)
  phase 2: per head pair: S^T[j,i] = kT^T qT (row-packed, 2 heads share the
           PE array), E = exp(S^T), A^T[65, i] += v_aug[j]^T E[j]
           row 64 of A^T = softmax denominators; normalize via reciprocal +
           gpsimd partition_broadcast + DVE multiply
  phase 3: out[i, m] = attn_stack^T Wo_g  (K=256 contraction over 2 tiles)

The 1/sqrt(64) score scale is folded into Wq/bq on the host.

v2 changes vs baseline (250877 ns):
  - block-major input DMAs in need-order across 4 queues so the critical
    3MB (wq+wk+xq blk0+xk blk0) lands first; first score MM ~20us earlier
  - scratch warm-up matmuls so the PE HAM clock-gate opens early
  - tail: normalize-chain DMAs moved off the sync queue (gpsimd/vector),
    staggered dummy MMs keep the PE warm through the chain, final i-block
    bounce copies alternate DVE/ACT and out-DMAs alternate sync/scalar

v3 changes (228071 -> ~213-215k ns measured on a healthy device):
  - v projection loses its bias row + ones row: softmax rows sum to 1, so
    the v-bias contribution to the output is the constant bv @ Wo -- folded
    into bo on the host.  The per-head ones channel of vt (softmax
    denominators) is written once by a strided memset instead of a [1,*]
    matmul per j-tile: kills 16 PE matmuls + the xvon/wvb DMAs.
  - vt copy is a single strided CAST (psum [128,256] -> 4x65-stride cols)
  - warm-up/bridge dummy matmuls switched to bf16 operands: the fp32 scr
    tile made each one lower to a ~1us LOW/HIGH instruction pair, blocking
    the in-order PE queue ~10us at the start and ~13us at the tail.
  - AV lag tapers over the last window so the post-exp drain is ~2 halves
    instead of 16, shortening the tail by ~8us.  (AV emission is in
    half-pair units; all drains stay AFTER the v_groups/exp that feed
    them -- a consumer emitted before its same-queue producer corrupts.)

Notes from measurement (for future work): the exp stream on the Scalar
engine (128 x ~1115ns = 143us serial) is the pacing resource in steady
state; PSUM's 8 banks block exp batching to [128,2048]; both HWDGE DMA
rings merge into one ~350GB/s stream (order matters, ring choice mostly
doesn't; SWDGE is ~40GB/s and inefficient); InstReciprocal costs ~6.5ns
per element-lane (use [128,4] layouts or reciprocal_approx_fast);
mid-kernel chains must not put work on the PE (it is ~90% busy).
Beware: this device drifts into a ~+35us slow state (P0 downclock?)
after ~1.5h of sustained runs and recovers after idling -- A/B timing
comparisons are only valid within a healthy window.

v5 changes (-> ~210-212k ns): wk moved from SWDGE to the scalar HWDGE
ring head with xk0/xq0-lo in 2-k-tile chunks, and the kT0 group emitted
BEFORE qT0 (its inputs land first).  Mid-stream exp gaps dropped ~5us
(the window-0 boundary stall 10us -> 6us).  Respreading qk lumps into
neighboring windows' slots measured neutral again -- the remaining
boundary stalls are PE/DMA-throughput-bound, not emission-order-bound.
Also measured neutral: kT0/qT0 half-group interleave in DMA-arrival
order, xq0-hi chunking, wq-gated warmups beyond 3 (sharply negative at
9 -- they delay the kT0 chain in the in-order queue).  Eight straight
neutral-or-worse micro-experiments before v6: emission-order shuffles
don't pay; only ARRIVAL-TIME alignment does.

v6 changes (-> ~209.5k ns): computed the DMA arrival curve (~2.87us/MB
after a 12us ramp) against each consumer's deadline: xq1 sat ~9us early
in the stream while xv1-3 ran 2.5-3.3us late for the window-0 tail
v_groups.  Demoted xq1 past xv2, and moved the chunk-3 v_groups
(v12-15) into window 1 slots 1-4 (their drain consumers AV(0,12..15)
fire at win1 s4-7, so emission order stays producer-first).
"""

import numpy as np
import ml_dtypes
from contextlib import ExitStack

import concourse.bass as bass
import concourse.tile as tile
from concourse import bacc, mybir
from concourse.bass_utils import run_bass_kernel_spmd

F32 = mybir.dt.float32
BF16 = mybir.dt.bfloat16
AF = mybir.ActivationFunctionType

D_MODEL = 1024
NUM_HEADS = 16
DK = 64
B = 2
S = 2048
NG = 4  # head groups = cores per batch
HPG = 4  # heads per group
CG = HPG * DK  # 256 channels per group
VW = HPG * (DK + 1)  # 260: per-head [v_h | ones]
VC = HPG * DK  # 256: v projection channels (no ones)
IB = 512  # i-block (query) width
NI = S // IB
NJ = S // 128
NK = D_MODEL // 128
BW = NK * IB  # 4096: cols per x block in SBUF (block-major)

_CACHE = {}


def build_program():
    nc = bacc.Bacc("TRN2", target_bir_lowering=False, debug=False, num_devices=8)
    xq_d = nc.dram_tensor("xq", [D_MODEL, S], BF16, kind="ExternalInput")
    xk_d = nc.dram_tensor("xk", [D_MODEL, S], BF16, kind="ExternalInput")
    xv_d = nc.dram_tensor("xv", [D_MODEL, S], BF16, kind="ExternalInput")
    wq_d = nc.dram_tensor("wq", [D_MODEL, CG], BF16, kind="ExternalInput")
    wk_d = nc.dram_tensor("wk", [D_MODEL, CG], BF16, kind="ExternalInput")
    bqk_d = nc.dram_tensor("bqk", [128, 4], F32, kind="ExternalInput")
    wv_d = nc.dram_tensor("wv", [D_MODEL, VC], BF16, kind="ExternalInput")
    wo_d = nc.dram_tensor("wo", [CG, D_MODEL], BF16, kind="ExternalInput")
    out_d = nc.dram_tensor("out", [S, D_MODEL], BF16, kind="ExternalOutput")

    with tile.TileContext(nc) as tc, ExitStack() as ctx:
        wpool = ctx.enter_context(tc.tile_pool(name="wpool", bufs=1))
        xpool = ctx.enter_context(tc.tile_pool(name="xpool", bufs=1))
        qkvpool = ctx.enter_context(tc.tile_pool(name="qkv", bufs=1))
        attnpool = ctx.enter_context(tc.tile_pool(name="attn", bufs=1))
        spsum = ctx.enter_context(tc.tile_pool(name="spsum", bufs=2, space="PSUM"))
        apsum = ctx.enter_context(tc.tile_pool(name="apsum", bufs=1, space="PSUM"))
        epool = ctx.enter_context(tc.tile_pool(name="epool", bufs=12))
        rpool = ctx.enter_context(tc.tile_pool(name="rpool", bufs=2))
        ph1psum = ctx.enter_context(tc.tile_pool(name="ph1psum", bufs=2, space="PSUM"))
        obounce = ctx.enter_context(tc.tile_pool(name="obounce", bufs=4))

        # ---- SBUF input tiles, block-major: x block b occupies cols
        # [b*BW, (b+1)*BW), k-tile k at +k*IB within it ----
        xq_sb = xpool.tile([128, NI * BW], BF16)
        xk_sb = xpool.tile([128, NI * BW], BF16)
        xv_sb = xpool.tile([128, 4 * BW], BF16)  # chunk c: j-cols [c*512,(c+1)*512)
        wq_sb = wpool.tile([128, NK * CG], BF16)  # k-tile k at cols [CG*k, CG*(k+1))
        wk_sb = wpool.tile([128, NK * CG], BF16)
        wv_sb = wpool.tile([128, NK * VC], BF16)
        bqk_sb = wpool.tile([128, 4], F32)  # cols: [q_cb0, k_cb0, q_cb1, k_cb1]
        wo_sb = [wpool.tile([128, D_MODEL], BF16, name=f"wo{t}") for t in range(2)]
        # warm-up matmul operands: MUST be bf16 -- fp32 matmuls lower to
        # LOW/HIGH instruction pairs at ~1us each and block the in-order PE
        # queue for ~10us
        scr = wpool.tile([1, 128 + IB], BF16)
        # den-broadcast matmul stationary: ones row at partition 64 so its
        # base partition matches the psum sums-row operand
        wones = wpool.tile([DK + 1, DK], BF16)

        def dma_w(eng, dst_sb, src_d, cb, k0, k1):
            # k-tiles [k0,k1) of column-block cb of a [D_MODEL, CG] weight
            eng.dma_start(
                dst_sb[:, cb * NK * 128 + k0 * 128 : cb * NK * 128 + k1 * 128],
                src_d.ap()[k0 * 128 : k1 * 128, cb * 128 : (cb + 1) * 128].rearrange(
                    "(n p) m -> p n m", p=128
                ),
            )

        def dma_xblk(eng, dst_sb, src_d, blk, k0=0, k1=NK):
            # [k1-k0 k-tiles, IB cols] of block blk -> block-major SBUF
            eng.dma_start(
                dst_sb[:, blk * BW + k0 * IB : blk * BW + k1 * IB],
                src_d.ap()[k0 * 128 : k1 * 128, blk * IB : (blk + 1) * IB].rearrange(
                    "(n p) m -> p n m", p=128
                ),
            )

        # ---- input DMA plan: 3 rings (only SP/ACT/gpsimd can issue DMAs).
        # The HWDGE keeps ~4-5 ring entries in flight CONCURRENTLY sharing
        # the ring bandwidth, so BOTH HWDGE rings (sync + scalar) carry only
        # critical-path bytes first, chunked at 2 k-tiles so semaphores fire
        # progressively and the first qT/kT groups pipeline with the DMA.
        # The gpsimd SWDGE ring is ~4x slower -- bulk/late-need only. ----
        nc.gpsimd.memset(scr[:], 0)
        nc.gpsimd.memset(wones[DK : DK + 1, :], 1.0)

        # sync ring: strict need-order.  The HWDGE keeps ~4 transfers of a
        # ring in flight CONCURRENTLY, so only the first ~4 items per ring
        # should be critical-path bytes; the bulk queues behind them.
        nc.sync.dma_start(wq_sb[:], wq_d.ap().rearrange("(n p) m -> p n m", p=128))
        dma_xblk(nc.sync, xq_sb, xq_d, 0, 4, 8)
        dma_xblk(nc.sync, xk_sb, xk_d, 1)
        dma_xblk(nc.sync, xk_sb, xk_d, 2)
        dma_xblk(nc.sync, xv_sb, xv_d, 0, 0, 8)
        dma_xblk(nc.sync, xk_sb, xk_d, 3)
        dma_xblk(nc.sync, xv_sb, xv_d, 1, 0, 8)
        dma_xblk(nc.sync, xv_sb, xv_d, 2, 0, 8)
        dma_xblk(nc.sync, xq_sb, xq_d, 1)
        dma_xblk(nc.sync, xv_sb, xv_d, 3, 0, 8)
        dma_xblk(nc.sync, xq_sb, xq_d, 2)
        dma_xblk(nc.sync, xq_sb, xq_d, 3)
        for t in range(2):
            nc.sync.dma_start(wo_sb[t][:], wo_d.ap()[t * 128 : (t + 1) * 128, :])
        # scalar ring: wk first (the kT stationaries gate the first score
        # pair; on the fast HWDGE pipe it lands ~12us instead of ~20 on
        # SWDGE), then xk0/xq0 low k-tiles in 2-tile chunks so the kT0
        # chain pipelines with the DMA arrivals
        nc.scalar.dma_start(wk_sb[:], wk_d.ap().rearrange("(n p) m -> p n m", p=128))
        dma_xblk(nc.scalar, xk_sb, xk_d, 0, 0, 2)
        dma_xblk(nc.scalar, xk_sb, xk_d, 0, 2, 4)
        dma_xblk(nc.scalar, xq_sb, xq_d, 0, 0, 2)
        dma_xblk(nc.scalar, xq_sb, xq_d, 0, 2, 4)
        # gpsimd ring: small/critical only
        nc.gpsimd.dma_start(bqk_sb[:], bqk_d.ap())
        dma_xblk(nc.gpsimd, xk_sb, xk_d, 0, 4, 8)
        nc.gpsimd.dma_start(
            wv_sb[:], wv_d.ap().rearrange("(n p) m -> p n m", p=128)
        )

        # ---- PE warm-up: garbage matmuls so the HAM clock-gate opens
        # before real work arrives (results never read) ----
        def dummy_mm(n=128):
            # small free dim: enough PE activity for the HAM window without
            # stealing real streaming cycles (~60ns each warm)
            pd = ph1psum.tile([128, IB], F32, name="p0")
            nc.tensor.matmul(
                pd[:, 0:n], scr[:, 0:128], scr[:, 128 : 128 + n],
                start=True, stop=True,
            )

        for _ in range(5):
            dummy_mm(IB)

        # HAM bridge: free warmups end ~3us before the first weight bytes
        # land, so the clock-gate re-throttles.  These wait for the wq DMA
        # and run right before the first real qT group.
        def dummy_wq():
            pd = ph1psum.tile([128, IB], F32, name="p0")
            nc.tensor.matmul(
                pd[:, 0:IB], scr[:, 0:128], wq_sb[0:1, 0:IB],
                start=True, stop=True,
            )

        for _ in range(3):
            dummy_wq()

        # ---- phase 1 outputs: per-block tiles for fine-grained overlap ----
        qTt = [[qkvpool.tile([128, IB], BF16, name=f"qT{cb}_{i}") for i in range(NI)]
               for cb in range(2)]
        kTt = [[qkvpool.tile([128, IB], BF16, name=f"kT{cb}_{i}") for i in range(NI)]
               for cb in range(2)]
        vt = [qkvpool.tile([128, VW], BF16, name=f"v{j}") for j in range(NJ)]
        attt = [[attnpool.tile([128, IB], BF16, name=f"att{t}_{i}") for i in range(NI)]
                for t in range(2)]

        # per-head ones channel (softmax denominator accumulator): written
        # once up front, never touched by the vt CASTs (they skip col 64 of
        # each head's 65-col band)
        for j in range(NJ):
            nc.vector.memset(vt[j][:, DK : VW : DK + 1], 1.0)

        qk_live = {}

        def qk_part(cb, tsel, i, k0, k1):
            """partial projection accumulation (k-tiles [k0,k1)) for
            qTt/kTt[cb][i]; the psum tile persists across parts so a group
            can be spread over several slots"""
            xt, w_sb, dst, bcol = (
                (xq_sb, wq_sb, qTt, 0) if tsel == 0 else (xk_sb, wk_sb, kTt, 1)
            )
            key = (cb, tsel, i)
            if key not in qk_live:
                qk_live[key] = ph1psum.tile([128, IB], F32, name="p0")
            pq = qk_live[key]
            for k in range(k0, k1):
                nc.tensor.matmul(
                    pq[:],
                    w_sb[:, k * CG + cb * 128 : k * CG + (cb + 1) * 128],
                    xt[:, i * BW + k * IB : i * BW + (k + 1) * IB],
                    start=(k == 0),
                    stop=(k == NK - 1),
                )
            if k1 == NK:
                del qk_live[key]
                nc.vector.tensor_scalar_add(
                    dst[cb][i][:], pq[:], bqk_sb[:, 2 * cb + bcol : 2 * cb + bcol + 1]
                )

        def qk_group(cb, tsel, i):
            qk_part(cb, tsel, i, 0, NK)

        def v_group(j):
            """v[j, e] = sum_k xv[k, j] wv[k, e] (bias folded into bo on host)"""
            pv = ph1psum.tile([128, VC], F32, name="p0")
            c, sub = j // 4, j % 4
            for k in range(NK):
                nc.tensor.matmul(
                    pv[:],
                    xv_sb[:, c * BW + k * IB + sub * 128 : c * BW + k * IB + (sub + 1) * 128],
                    wv_sb[:, k * VC : (k + 1) * VC],
                    start=(k == 0),
                    stop=(k == NK - 1),
                )
            # strided CAST: psum head-h cols [h*64,(h+1)*64) -> vt cols at
            # stride 65, skipping the ones channel
            nc.vector.tensor_copy(
                vt[j][:].rearrange("p (h c) -> p h c", h=HPG)[:, :, 0:DK],
                pv[:].rearrange("p (h c) -> p h c", h=HPG),
            )

        _sid1, _ = nc.enter_named_scope("phase1", False)
        qk_group(0, 1, 0)  # kT block 0 first: wk/xk0-lo land before wq/xq0-hi
        qk_group(0, 0, 0)
        nc.leave_named_scope("phase1", _sid1, False)

        # ---- phase 2 (with v-proj pipelined into pr0/i0, cb1 into i1/i2,
        # and phase 3 pipelined into pr1) ----
        _sid2, _ = nc.enter_named_scope("phase2", False)
        windows = [(pr, i) for pr in range(2) for i in range(NI)]
        LAG = 8  # AV trails exp by this many j-slots (e-pool is the buffer)
        pending = []  # (wi, j, ee)
        aa_cur = {}
        aa_of = {}

        def emit_av_half(wi, j, ee, u):
            """one head's AV matmul; u=0 is emitted before the current score
            pair and u=1 after, so the pair's kT weight loads hide under the
            u=0 stream"""
            pr, i = windows[wi]
            if j == 0 and u == 0:
                for uu in range(2):
                    aa_cur[uu] = apsum.tile([DK + 1, IB], F32, name=f"a{uu}")
                aa_of[wi] = dict(aa_cur)
            h = 2 * pr + u
            nc.tensor.matmul(
                aa_of[wi][u][:],
                vt[j][:, h * (DK + 1) : (h + 1) * (DK + 1)],
                ee[:, u * IB : (u + 1) * IB],
                start=(j == 0),
                stop=(j == NJ - 1),
            )
            if j == NJ - 1 and u == 1:
                finish_window(wi)

        def finish_window(wi):
            pr, i = windows[wi]
            last = wi == len(windows) - 1
            aa = aa_of.pop(wi)
            if wi == len(windows) - 2:
                # bridge the normalize-chain PE idle so HAM stays warm
                for _ in range(4):
                    dummy_mm()
            if last:
                # tail fast chain: broadcast each sums row across the 64
                # channel partitions with a [1,64] ones outer-product matmul
                # (216ns, PE is idle here) instead of 2 DMA hops + gpsimd;
                # approx-reciprocal lane-aligned from psum.  Dummy matmuls
                # gated on chain stages keep the HAM clock open so the
                # phase-3 matmuls run warm.
                cd = {}
                for u in (1, 0):
                    cd[u] = rpool.tile([DK + 1, IB], BF16, name="cden")
                    nc.vector.tensor_copy(
                        cd[u][DK : DK + 1, :], aa[u][DK : DK + 1, :]
                    )
                dummy_mm(IB)
                rbs = {}
                for u in (1, 0):
                    rbp = ph1psum.tile([DK, IB], F32, name="p0")
                    nc.tensor.matmul(
                        rbp[:], wones[DK : DK + 1, :], cd[u][DK : DK + 1, :],
                        start=True, stop=True,
                    )
                    rbs[u] = rpool.tile([DK, IB], F32, name="rb")
                    nc.vector.reciprocal_approx_fast(out=rbs[u][:], in_=rbp[:])
                nrm = rpool.tile([DK, IB], BF16, name="nrm")
                nc.vector.tensor_mul(nrm[:], aa[1][0:DK, :], rbs[1][:])
                nc.scalar.dma_start(
                    attt[1][i][64 : 64 + DK, :], nrm[:]
                )
                def dummy_gated(rhs_ap):
                    pd = ph1psum.tile([128, IB], F32, name="p0")
                    nc.tensor.matmul(
                        pd[:, 0:IB], scr[:, 0:128], rhs_ap, start=True, stop=True
                    )
                dummy_gated(nrm[0:1, :])
                nc.vector.tensor_mul(attt[1][i][0:DK, :], aa[0][0:DK, :], rbs[0][:])
                dummy_gated(attt[1][i][0:1, 0:IB])
            else:
              for u in range(2):
                h = 2 * pr + u
                # copy A^T out of PSUM fast so the bank frees; sums row
                # (partition 64) -> [128, 4] by DMA so the reciprocal runs
                # 4 elems/lane instead of 512
                asb = rpool.tile([DK + 1, IB], F32, name="asb")
                nc.vector.tensor_copy(asb[:], aa[u][:])
                ceng = nc.sync
                r4 = rpool.tile([128, 4], F32, name="r4")
                ceng.dma_start(r4[:], asb[DK : DK + 1, :])
                r4b = rpool.tile([128, 4], F32, name="r4b")
                nc.vector.reciprocal(r4b[:], r4[:])
                r0 = rpool.tile([1, IB], F32, name="r0")
                ceng.dma_start(r0[:], r4b[:])
                rb = rpool.tile([DK, IB], F32, name="rb")
                nc.gpsimd.partition_broadcast(rb[:], r0[:])
                t, po = h // 2, 64 * (h % 2)
                if po == 0:
                    nc.vector.tensor_mul(attt[t][i][0:DK, :], asb[0:DK, :], rb[:])
                else:
                    nrm = rpool.tile([DK, IB], BF16, name="nrm")
                    nc.vector.tensor_mul(nrm[:], asb[0:DK, :], rb[:])
                    ceng.dma_start(attt[t][i][po : po + DK, :], nrm[:])
            if pr == 1:
                # phase 3 for this i-block: all four heads' attn now ready.
                late = wi >= len(windows) - 2
                for ibl in range(4):
                    ib = i * 4 + ibl
                    for mh in range(2):
                        po_t = ph1psum.tile([128, IB], F32, name="p0")
                        for t in range(2):
                            nc.tensor.matmul(
                                po_t[:],
                                attt[t][i][:, ibl * 128 : (ibl + 1) * 128],
                                wo_sb[t][:, mh * IB : (mh + 1) * IB],
                                start=(t == 0),
                                stop=(t == 1),
                            )
                        ob = obounce.tile([128, IB], BF16, name="ob")
                        if late and (ib + mh) % 2 == 0:
                            nc.scalar.activation(ob[:], po_t[:], AF.Identity)
                        else:
                            nc.vector.tensor_copy(ob[:], po_t[:])
                        oeng = nc.scalar if (last and (ib + mh) % 2 == 1) else nc.sync
                        oeng.dma_start(
                            out_d.ap()[ib * 128 : (ib + 1) * 128,
                                       mh * IB : (mh + 1) * IB],
                            ob[:],
                        )

        halves = []  # (wi, j, ee, u) AV work units, emitted one per half-slot

        def drain_halves(n):
            for _ in range(min(n, len(halves))):
                emit_av_half(*halves.pop(0))

        # qk work spread as 2-3 k-tile parts across slots where the p0 psum
        # pool is otherwise idle, so no window-boundary 8-MM lump ever
        # starves the exp stream.  (cb, tsel, i, k0, k1) at (wi, j):
        for wi, (pr, i) in enumerate(windows):
            last_win = wi == len(windows) - 1
            if wi > 0 and pr == 0:
                qk_group(0, 0, i)  # q-block for this window's rhs
            elif wi in (4, 5, 6):
                # qT cb1 blocks 1-3 into the second half; block wi-3 is
                # consumed by window wi+1
                qk_group(1, 0, wi - 3)
            for j in range(NJ):
                if wi == 0 and j % 4 == 0 and j > 0:
                    qk_group(0, 1, j // 4)  # k-block feeding this score quad
                elif wi == 2 and j % 4 == 0:
                    qk_group(1, 1, j // 4)  # kT cb1 blocks
                elif wi == 3 and j == 0:
                    qk_group(1, 0, 0)  # qT cb1 block 0 (needed at window 4)
                # AV lag: taper over the last window so the post-exp drain
                # is short
                lag_h = 2 * LAG
                if last_win:
                    lag_h = max(2, 2 * LAG - 2 * max(0, j - 5))
                # both heads' scores in one 2-bank psum tile -> one exp
                ss = spsum.tile([128, 2 * IB], F32, name="ss")
                for u in range(2):
                    nc.tensor.matmul(
                        ss[:, u * IB : (u + 1) * IB],
                        kTt[pr][j // 4][u * DK : (u + 1) * DK,
                                        (j % 4) * 128 : (j % 4 + 1) * 128],
                        qTt[pr][i][u * DK : (u + 1) * DK, :],
                        start=True,
                        stop=True,
                        tile_position=(u * DK, 0),
                    )
                ee = epool.tile([128, 2 * IB], BF16, name="ee")
                nc.scalar.activation(ee[:], ss[:], AF.Exp)
                halves.append((wi, j, ee, 0))
                halves.append((wi, j, ee, 1))
                if wi == 0 and 8 <= j <= 13:
                    for jj in (2 * (j - 8), 2 * (j - 8) + 1):
                        v_group(jj)
                elif wi == 1 and 1 <= j <= 4:
                    v_group(11 + j)
                while len(halves) > lag_h:
                    drain_halves(1)
        while halves:
            drain_halves(2)
            if len(halves) in (2, 6, 10):
                dummy_mm()  # keep the HAM clock open through the drain
        nc.leave_named_scope("phase2", _sid2, False)

    nc.compile()
    return nc


def _prep_inputs(Q, K, V, Wq, bq, Wk, bk, Wv, bv, Wo, bo):
    """Build the 8 per-core input maps (host-side shard + layout)."""
    bf16 = ml_dtypes.bfloat16
    per_batch = []
    for b in range(B):
        xq = np.ascontiguousarray(Q[b].T).astype(bf16)
        xk = np.ascontiguousarray(K[b].T).astype(bf16)
        xv = np.ascontiguousarray(V[b].T).astype(bf16)
        per_batch.append((xq, xk, xv))
    in_maps = []
    for c in range(8):
        b, g = divmod(c, NG)
        xq, xk, xv = per_batch[b]
        gs = slice(g * CG, (g + 1) * CG)
        wq = np.ascontiguousarray(Wq[:, gs]) * 0.125
        wk = np.ascontiguousarray(Wk[:, gs])
        bqs, bks = bq[gs] * 0.125, bk[gs]
        bqk = np.stack(
            [bqs[:128], bks[:128], bqs[128:], bks[128:]], axis=1
        ).astype(np.float32)
        wv = np.ascontiguousarray(Wv[:, gs])
        wo = np.ascontiguousarray(Wo[g * CG : (g + 1) * CG, :])
        in_maps.append(
            {
                "xq": xq,
                "xk": xk,
                "xv": xv,
                "wq": wq.astype(bf16),
                "wk": wk.astype(bf16),
                "bqk": bqk,
                "wv": wv.astype(bf16),
                "wo": wo.astype(bf16),
            }
        )
    return in_maps


def run(inputs, trace=False):
    if "nc" not in _CACHE:
        _CACHE["nc"] = build_program()
    nc = _CACHE["nc"]
    in_maps = _prep_inputs(**inputs)
    res = run_bass_kernel_spmd(nc, in_maps, core_ids=list(range(8)), trace=trace)
    # v bias folds into the output bias: softmax rows sum to 1, so the
    # device-side v projection omits bv and the host adds bv @ Wo here.
    bo = np.asarray(inputs["bo"], dtype=np.float32) + (
        np.asarray(inputs["bv"], dtype=np.float32)
        @ np.asarray(inputs["Wo"], dtype=np.float32)
    )
    outs = []
    for b in range(B):
        acc = res.results[4 * b]["out"].astype(np.float32)
        for g in range(1, NG):
            acc = acc + res.results[4 * b + g]["out"].astype(np.float32)
        outs.append(acc + bo[None, :])
    return np.stack(outs, axis=0), res


def kernel(**inputs):
    inputs = {k: np.asarray(v) for k, v in inputs.items()}
    out, _ = run(inputs, trace=False)
    return out.astype(np.float32)

